# revision 12
# baseline (speedup 1.0000x reference)
"""Trainium2 Bass kernel for AttentionBase (b=4, n=2048, h=8, d=64, F=512).

Sharding: 8 cores; core c handles batch b = c//2, query rows
i in [(c%2)*1024, (c%2)*1024 + 1024), all 8 heads. Each core's output slice
is independent -> no collectives; host gathers by concatenation.

Kernel strategy (per core):
  - Host pre-transposes everything so the device does only dense matmuls:
      biasT[h, j', i]  (mask folded in as -1e9, null-token column moved to
                        j' = 2048 so the regular range is exactly 16*128)
      qT[h, d, i] (pre-scaled by d^-0.5), kT[h, d, j'], vA[j', h*65]
      (v columns + a ones column per head for softmax row sums).
  - S^T[j,i] = K @ Q^T via matmul (contraction d=64, two heads packed per
    128-partition tile), bias added by accumulating I.T @ biasT into PSUM.
  - P^T = exp(S^T) on ACT straight out of PSUM (no max subtraction needed:
    logits are O(10); masked entries are -1e9 -> exp = 0 exactly).
  - PV: lhsT = [V | ones] (M=65) so row 64 of the PSUM result is the
    softmax denominator. Normalization happens on the X^T copy in SBUF.
  - Projection X @ W^T runs in natural [i, F] layout (lhsT = X^T tiles),
    then CenteredLayerNorm along the free dim.
All matmul operands are bitcast to float32r (fp32 bits, 1 cycle/row at
N>=512 on the PE) for speed; accumulation stays fp32 in PSUM.
"""

import os
import numpy as np
from contextlib import ExitStack

import concourse.bass as bass
import concourse.bacc as bacc
import concourse.tile as tile
import concourse.mybir as mybir
from concourse.bass_utils import run_bass_kernel_spmd

B, N, H, D = 4, 2048, 8, 64
MID = H * D  # 512
F = 512
NCORES = 8
NI = 1024  # query rows per core
JT = 16  # full 128-row j' tiles (regular tokens); null token handled apart
EPS = 1e-5
NEG = np.float32(-1e9)

F32 = mybir.dt.float32
F32R = mybir.dt.float32r
AX = mybir.AxisListType.X
ALU = mybir.AluOpType
ACTF = mybir.ActivationFunctionType

LAST_RESULT = None  # BassKernelResults of the most recent run (for test.py)
_NC_CACHE = {}


def _r(ap):
    """View an fp32 AP as float32r for the PE fast path (same bits)."""
    return ap.bitcast(F32R)


def _ensure_ntff_hook():
    """Register the axon NTFF profiling hook if the image lacks antenv.axon_hooks."""
    import sys
    import types

    try:
        from antenv.axon_hooks import get_axon_ntff_profile_hook  # noqa: F401
        return
    except ImportError:
        pass
    mod = types.ModuleType("antenv.axon_hooks")
    holder = {"h": None}
    mod.set_axon_ntff_profile_hook = lambda h: holder.__setitem__("h", h)
    mod.get_axon_ntff_profile_hook = lambda: holder["h"]
    import antenv

    sys.modules["antenv.axon_hooks"] = mod
    antenv.axon_hooks = mod
    try:
        from trn_agent_boot.trn_boot import _ntff_profile_via_ctypes

        h = _ntff_profile_via_ctypes("/opt/axon/libaxon_pjrt.so")
        if h is not None:
            mod.set_axon_ntff_profile_hook(h)
    except Exception:
        pass


def build_nc():
    nc = bacc.Bacc()
    biasT = nc.declare_dram_parameter("biasT", [H, N + 1, NI], F32R, isOutput=False)
    qT = nc.declare_dram_parameter("qT", [H, D, NI], F32R, isOutput=False)
    kT = nc.declare_dram_parameter("kT", [H, D, N + 1], F32R, isOutput=False)
    vA = nc.declare_dram_parameter("vA", [N + 1, H * 65], F32R, isOutput=False)
    wT = nc.declare_dram_parameter("wT", [MID, F], F32R, isOutput=False)
    gam = nc.declare_dram_parameter("gam", [128, F], F32, isOutput=False)
    ident = nc.declare_dram_parameter("ident", [128, 128], F32R, isOutput=False)
    e8 = nc.declare_dram_parameter("e8", [1, 64], F32R, isOutput=False)
    m8 = nc.declare_dram_parameter("m8", [4, 8, 128], F32R, isOutput=False)
    one1 = nc.declare_dram_parameter("one1", [1, 1], F32R, isOutput=False)
    outp = nc.declare_dram_parameter("out", [NI, F], F32, isOutput=True)

    with ExitStack() as ctx:
        tc = ctx.enter_context(tile.TileContext(nc))
        const = ctx.enter_context(tc.tile_pool(name="const", bufs=1))
        biasp = ctx.enter_context(tc.tile_pool(name="biasp", bufs=2))
        ptp = ctx.enter_context(tc.tile_pool(name="ptp", bufs=3))
        smalls = ctx.enter_context(tc.tile_pool(name="smalls", bufs=2))
        xtp = ctx.enter_context(tc.tile_pool(name="xtp", bufs=2))
        rrp = ctx.enter_context(tc.tile_pool(name="rrp", bufs=2))
        lnp = ctx.enter_context(tc.tile_pool(name="lnp", bufs=2))
        outpool = ctx.enter_context(tc.tile_pool(name="outpool", bufs=3))
        ps_s = ctx.enter_context(tc.tile_pool(name="ps_s", bufs=2, space="PSUM"))
        ps_pv = ctx.enter_context(tc.tile_pool(name="ps_pv", bufs=2, space="PSUM"))
        ps_rr = ctx.enter_context(tc.tile_pool(name="ps_rr", bufs=1, space="PSUM"))
        ps_pk = ctx.enter_context(tc.tile_pool(name="ps_pk", bufs=1, space="PSUM"))
        ps_pj = ctx.enter_context(tc.tile_pool(name="ps_pj", bufs=2, space="PSUM"))

        # ---- persistent tiles -------------------------------------------
        kT_sb, qT_sb, w_sb = [], [], []
        for m in range(4):  # head pair m -> heads 2m (parts 0-63), 2m+1 (64-127)
            kt = const.tile([128, N + 1], F32R, tag=f"kt{m}")
            nc.sync.dma_start(out=kt, in_=kT[2 * m : 2 * m + 2].rearrange("a b c -> (a b) c"))
            kT_sb.append(kt)
            qt = const.tile([128, NI], F32R, tag=f"qt{m}")
            nc.sync.dma_start(out=qt, in_=qT[2 * m : 2 * m + 2].rearrange("a b c -> (a b) c"))
            qT_sb.append(qt)
            w = const.tile([128, F], F32R, tag=f"w{m}")
            nc.sync.dma_start(out=w, in_=wT[m * 128 : (m + 1) * 128, :])
            w_sb.append(w)
        vA_sb = const.tile([128, JT * H * 65], F32R, tag="vA")
        nc.sync.dma_start(
            out=vA_sb[:, :].rearrange("p (a c) -> p a c", a=JT),
            in_=vA[0 : JT * 128, :].rearrange("(a p) c -> p a c", p=128),
        )
        vnull = const.tile([1, H * 65], F32R, tag="vnull")
        nc.sync.dma_start(out=vnull, in_=vA[N : N + 1, :])
        gam_sb = const.tile([128, F], F32, tag="gam")
        nc.sync.dma_start(out=gam_sb, in_=gam[:, :])
        id_sb = const.tile([128, 128], F32R, tag="ident")
        nc.sync.dma_start(out=id_sb, in_=ident[:, :])
        e8_sb = const.tile([1, 64], F32R, tag="e8")
        nc.sync.dma_start(out=e8_sb, in_=e8[:, :])
        m8_sb = const.tile([8, 4 * 128], F32R, tag="m8")
        nc.sync.dma_start(
            out=m8_sb[:, :].rearrange("p (a c) -> p a c", a=4),
            in_=m8[:, :, :].rearrange("a p c -> p a c"),
        )
        one11 = const.tile([1, 1], F32R, tag="one11")
        nc.sync.dma_start(out=one11, in_=one1[:, :])

        # ---- main loop ---------------------------------------------------
        for ib in range(2):
            i0 = ib * 512
            xts = [xtp.tile([128, 512], F32R, tag=f"xt{m}", name=f"xt{m}_{ib}") for m in range(4)]
            pk = ps_pk.tile([8, 512], F32, tag="pk")  # packed softmax sums
            for m in range(4):
                for hh in range(2):
                    h = 2 * m + hh
                    hs = slice(hh * 64, hh * 64 + 64)
                    pv = ps_pv.tile([65, 512], F32, tag="pv")
                    for jtg in range(2):
                        # 1 MB bias block: 8 j-tiles x 128 rows x 512 i-cols
                        bsb = biasp.tile([128, 8 * 512], F32R, tag="bias")
                        src = biasT[
                            h, jtg * 1024 : (jtg + 1) * 1024, i0 : i0 + 512
                        ].rearrange("(a p) f -> p a f", p=128)
                        dst = bsb[:, :].rearrange("p (a f) -> p a f", a=8)
                        nc.sync.dma_start(out=dst, in_=src)
                        for jj in range(8):
                            jt = jtg * 8 + jj
                            sp = ps_s.tile([128, 512], F32, tag="sp")
                            nc.tensor.matmul(
                                sp,
                                lhsT=kT_sb[m][hs, jt * 128 : (jt + 1) * 128],
                                rhs=qT_sb[m][hs, i0 : i0 + 512],
                                start=True,
                                stop=False,
                            )
                            nc.tensor.matmul(
                                sp,
                                lhsT=id_sb,
                                rhs=bsb[:, jj * 512 : (jj + 1) * 512],
                                start=False,
                                stop=True,
                            )
                            pt = ptp.tile([128, 512], F32R, tag="pt")
                            nc.scalar.activation(pt, sp, ACTF.Exp)
                            nc.tensor.matmul(
                                pv,
                                lhsT=vA_sb[:, (jt * H + h) * 65 : (jt * H + h + 1) * 65],
                                rhs=pt,
                                start=(jt == 0),
                                stop=False,
                            )
                    # null token (j' = 2048)
                    sn = ps_s.tile([1, 512], F32, tag="sp")
                    nc.tensor.matmul(
                        sn,
                        lhsT=kT_sb[m][hs, N : N + 1],
                        rhs=qT_sb[m][hs, i0 : i0 + 512],
                        start=True,
                        stop=False,
                    )
                    nb = smalls.tile([1, 512], F32R, tag="nb")
                    nc.sync.dma_start(out=nb, in_=biasT[h, N : N + 1, i0 : i0 + 512])
                    nc.tensor.matmul(sn, lhsT=one11, rhs=nb, start=False, stop=True)
                    ptn = smalls.tile([1, 512], F32R, tag="ptn")
                    nc.scalar.activation(ptn, sn, ACTF.Exp)
                    nc.tensor.matmul(
                        pv,
                        lhsT=vnull[0:1, h * 65 : (h + 1) * 65],
                        rhs=ptn,
                        start=False,
                        stop=True,
                    )
                    # stash unnormalized X^T rows + softmax sums
                    nc.vector.tensor_copy(xts[m][hs, :], pv[0:64, :])
                    ssb = smalls.tile([1, 512], F32R, tag="ssb")
                    nc.vector.tensor_copy(ssb, pv[64:65, :])
                    nc.tensor.matmul(
                        pk,
                        lhsT=e8_sb[0:1, h * 8 : h * 8 + 8],
                        rhs=ssb[:, :],
                        start=(h == 0),
                        stop=(h == 7),
                    )
            # 1/sums for all 8 heads at once, then broadcast via matmul
            pks = smalls.tile([8, 512], F32, tag="pks")
            nc.vector.tensor_copy(pks, pk)
            rca = smalls.tile([8, 512], F32R, tag="rca")
            with nc.allow_low_precision(reason="float32r has fp32 width"):
                nc.vector.reciprocal(rca, pks)
            for m in range(4):
                rr_ps = ps_rr.tile([128, 512], F32, tag="rr")
                nc.tensor.matmul(
                    rr_ps,
                    lhsT=m8_sb[:, m * 128 : (m + 1) * 128],
                    rhs=rca[:, :],
                    start=True,
                    stop=True,
                )
                rr_sb = rrp.tile([128, 512], F32, tag="rr_sb")
                nc.vector.tensor_copy(rr_sb, rr_ps)
                nc.vector.tensor_mul(xts[m], xts[m], rr_sb)
            # projection + CenteredLayerNorm per 128-row tile
            for it in range(4):
                pp = ps_pj.tile([128, 512], F32, tag="pp")
                for m in range(4):
                    nc.tensor.matmul(
                        pp,
                        lhsT=xts[m][:, it * 128 : (it + 1) * 128],
                        rhs=w_sb[m],
                        start=(m == 0),
                        stop=(m == 3),
                    )
                s1 = smalls.tile([128, 1], F32, tag="s1")
                nc.vector.reduce_sum(s1, pp, axis=AX)
                mu = smalls.tile([128, 1], F32, tag="mu")
                nc.vector.tensor_scalar_mul(mu, s1, 1.0 / F)
                cen = lnp.tile([128, 512], F32, tag="cen")
                nc.vector.tensor_scalar(
                    out=cen, in0=pp, scalar1=mu, scalar2=None, op0=ALU.subtract
                )
                sq = lnp.tile([128, 512], F32, tag="sq")
                var = smalls.tile([128, 1], F32, tag="var")
                nc.scalar.activation(sq, cen, ACTF.Square, accum_out=var)
                v2 = smalls.tile([128, 1], F32, tag="v2")
                nc.vector.tensor_scalar(
                    out=v2,
                    in0=var,
                    scalar1=1.0 / F,
                    scalar2=EPS,
                    op0=ALU.mult,
                    op1=ALU.add,
                )
                lnv = smalls.tile([128, 1], F32, tag="lnv")
                nc.scalar.activation(lnv, v2, ACTF.Ln)
                rstd = smalls.tile([128, 1], F32, tag="rstd")
                nc.scalar.activation(rstd, lnv, ACTF.Exp, scale=-0.5)
                o1 = lnp.tile([128, 512], F32, tag="o1")
                nc.vector.tensor_scalar_mul(o1, cen, rstd)
                o2 = outpool.tile([128, 512], F32, tag="o2")
                nc.vector.tensor_mul(o2, o1, gam_sb)
                nc.sync.dma_start(
                    out=outp[i0 + it * 128 : i0 + (it + 1) * 128, :], in_=o2
                )
    nc.finalize()
    return nc


def _host_prep(q, k, v, mask, bias, tokens, w_out, gamma):
    """Build the 8 per-core input maps (all plain numpy)."""
    wTc = np.ascontiguousarray(w_out.T)  # [MID, F]
    gam_rep = np.ascontiguousarray(np.broadcast_to(gamma[None, :], (128, F)))
    ident = np.eye(128, dtype=np.float32)
    e8 = np.eye(8, dtype=np.float32).reshape(1, 64)
    m8 = np.zeros((4, 8, 128), np.float32)
    for m in range(4):
        m8[m, 2 * m, :64] = 1.0
        m8[m, 2 * m + 1, 64:] = 1.0

    in_maps = [None] * NCORES
    for b in range(B):
        ka = np.concatenate([k[b], np.tile(tokens[0], H)[None, :]], axis=0)  # [N+1, MID]
        kTb = np.ascontiguousarray(ka.reshape(N + 1, H, D).transpose(1, 2, 0))
        va = np.concatenate([v[b], np.tile(tokens[1], H)[None, :]], axis=0)
        vAb = np.ascontiguousarray(
            np.concatenate(
                [va.reshape(N + 1, H, D), np.ones((N + 1, H, 1), np.float32)], axis=2
            ).reshape(N + 1, H * 65)
        )
        maskb = np.concatenate([mask[b], [True]])  # j' order: tokens..., null
        # reorder bias j: null column (orig j=0) moved to the end
        bb = np.concatenate([bias[b, :, :, 1:], bias[b, :, :, 0:1]], axis=2)
        bb = np.where(maskb[None, None, :], bb, NEG)  # [H, N(i), N+1(j')]
        bbT = bb.transpose(0, 2, 1)  # [H, N+1, N] (view)
        for half in range(2):
            c = 2 * b + half
            i0 = half * NI
            qTc = (
                q[b, i0 : i0 + NI].reshape(NI, H, D).transpose(1, 2, 0) / 8.0
            )  # d^-0.5 folded in
            in_maps[c] = {
                "biasT": np.ascontiguousarray(bbT[:, :, i0 : i0 + NI]),
                "qT": np.ascontiguousarray(qTc),
                "kT": kTb,
                "vA": vAb,
                "wT": wTc,
                "gam": gam_rep,
                "ident": ident,
                "e8": e8,
                "m8": m8,
                "one1": np.ones((1, 1), np.float32),
            }
    return in_maps


def kernel(q, k, v, mask, attention_bias, tokens, w_out, gamma):
    global LAST_RESULT
    q = np.asarray(q, np.float32)
    k = np.asarray(k, np.float32)
    v = np.asarray(v, np.float32)
    mask = np.asarray(mask, bool)
    bias = np.asarray(attention_bias, np.float32)
    tokens = np.asarray(tokens, np.float32)
    w_out = np.asarray(w_out, np.float32)
    gamma = np.asarray(gamma, np.float32)

    if "nc" not in _NC_CACHE:
        _NC_CACHE["nc"] = build_nc()
    nc = _NC_CACHE["nc"]

    in_maps = _host_prep(q, k, v, mask, bias, tokens, w_out, gamma)
    trace = os.environ.get("KERNEL_TRACE", "0") == "1"
    if trace:
        _ensure_ntff_hook()
        try:
            res = run_bass_kernel_spmd(nc, in_maps, list(range(NCORES)), trace=True)
        except Exception as e:
            print(f"trace run failed ({type(e).__name__}: {e}); retrying untraced")
            res = run_bass_kernel_spmd(nc, in_maps, list(range(NCORES)), trace=False)
    else:
        res = run_bass_kernel_spmd(nc, in_maps, list(range(NCORES)), trace=False)
    LAST_RESULT = res

    out = np.empty((B, N, F), np.float32)
    for c in range(NCORES):
        out[c // 2, (c % 2) * NI : (c % 2) * NI + NI, :] = res.results[c]["out"]
    return out


# revision 13
# speedup vs baseline: 1.7088x; 1.7088x over previous
"""Trainium2 Bass kernel for AttentionBase (b=4, n=2048, h=8, d=64, F=512).

Sharding: 8 cores; core c handles batch b = c//2, query rows
i in [(c%2)*1024, (c%2)*1024 + 1024), all 8 heads. Each core's output slice
is independent -> no collectives; host gathers by concatenation.

v2 design (per core):
  - Host pre-transposes/casts everything so the device does only dense
    1-cycle/row matmuls: biasT[h, j', i] fp16 (mask folded as -20000,
    null-token column moved to j' = 2048), qT[h, d, i] fp16 (pre-scaled by
    d^-0.5), kT[h, d, j'] fp16, vA[j', h*65] bf16 (v columns + ones column
    per head for softmax row sums), wT fp16.
  - S^T[j,i] = K @ Q^T (contraction d=64, 2 heads packed per 128-partition
    tile), bias added by accumulating I.T @ biasT into PSUM.
  - P^T = exp(S^T) on ACT straight from PSUM into bf16 (no max subtraction:
    logits are O(12); bf16 has fp32 range so exp never overflows; masked
    entries are -20000 -> exp = 0 exactly).
  - PV: lhsT = [V | ones] bf16 (M=65) so PSUM row 64 is the softmax
    denominator. Row sums of all heads are DMA-gathered into one [8,1024]
    tile for a single batched DVE reciprocal; 1/sums are broadcast across
    partitions with a small selector matmul and applied on the X^T copy.
  - Projection X @ W^T in natural [i, F] layout, then CenteredLayerNorm
    along the free dim (rstd via exp(-0.5*ln(var)) to stay in one ACT
    table set with exp).
  PSUM: 2 x [128,1024] S tiles (4 banks) + 4 shared [128,512] slots
  (PV halves / broadcast / projection) = 8 banks exactly.
"""

import os
import numpy as np
from contextlib import ExitStack

import ml_dtypes
import concourse.bass as bass
import concourse.bacc as bacc
import concourse.tile as tile
import concourse.mybir as mybir
from concourse.bass_utils import run_bass_kernel_spmd

B, N, H, D = 4, 2048, 8, 64
MID = H * D  # 512
F = 512
NCORES = 8
NI = 1024  # query rows per core
JT = 16  # full 128-row j' tiles (regular tokens); null token handled apart
EPS = 1e-5
NEG = np.float32(-20000.0)

F32 = mybir.dt.float32
F16 = mybir.dt.float16
BF16 = mybir.dt.bfloat16
AX = mybir.AxisListType.X
ALU = mybir.AluOpType
ACTF = mybir.ActivationFunctionType

LAST_RESULT = None  # BassKernelResults of the most recent run (for test.py)
_NC_CACHE = {}


def _ensure_ntff_hook():
    """Register the axon NTFF profiling hook if the image lacks antenv.axon_hooks."""
    import sys
    import types

    try:
        from antenv.axon_hooks import get_axon_ntff_profile_hook  # noqa: F401

        return
    except ImportError:
        pass
    mod = types.ModuleType("antenv.axon_hooks")
    holder = {"h": None}
    mod.set_axon_ntff_profile_hook = lambda h: holder.__setitem__("h", h)
    mod.get_axon_ntff_profile_hook = lambda: holder["h"]
    import antenv

    sys.modules["antenv.axon_hooks"] = mod
    antenv.axon_hooks = mod
    try:
        from trn_agent_boot.trn_boot import _ntff_profile_via_ctypes

        h = _ntff_profile_via_ctypes("/opt/axon/libaxon_pjrt.so")
        if h is not None:
            mod.set_axon_ntff_profile_hook(h)
    except Exception:
        pass


def build_nc():
    nc = bacc.Bacc()
    biasT = nc.declare_dram_parameter("biasT", [H, N + 1, NI], F16, isOutput=False)
    qT = nc.declare_dram_parameter("qT", [H, D, NI], F16, isOutput=False)
    kT = nc.declare_dram_parameter("kT", [H, D, N + 1], F16, isOutput=False)
    vA = nc.declare_dram_parameter("vA", [N + 1, H * 65], BF16, isOutput=False)
    wT = nc.declare_dram_parameter("wT", [MID, F], F16, isOutput=False)
    gam = nc.declare_dram_parameter("gam", [128, F], F32, isOutput=False)
    ident = nc.declare_dram_parameter("ident", [128, 128], F16, isOutput=False)
    m8 = nc.declare_dram_parameter("m8", [4, 8, 128], F16, isOutput=False)
    one1 = nc.declare_dram_parameter("one1", [1, 1], F16, isOutput=False)
    outp = nc.declare_dram_parameter("out", [NI, F], F32, isOutput=True)

    with ExitStack() as ctx:
        tc = ctx.enter_context(tile.TileContext(nc))
        const = ctx.enter_context(tc.tile_pool(name="const", bufs=1))
        biasp = ctx.enter_context(tc.tile_pool(name="biasp", bufs=2))
        ptp = ctx.enter_context(tc.tile_pool(name="ptp", bufs=3))
        smalls = ctx.enter_context(tc.tile_pool(name="smalls", bufs=2))
        xtp = ctx.enter_context(tc.tile_pool(name="xtp", bufs=1))
        rrp = ctx.enter_context(tc.tile_pool(name="rrp", bufs=2))
        lnp = ctx.enter_context(tc.tile_pool(name="lnp", bufs=2))
        outpool = ctx.enter_context(tc.tile_pool(name="outpool", bufs=3))
        ps_s = ctx.enter_context(tc.tile_pool(name="ps_s", bufs=2, space="PSUM"))
        ps_misc = ctx.enter_context(tc.tile_pool(name="ps_misc", bufs=4, space="PSUM"))

        # ---- persistent tiles -------------------------------------------
        kT_sb, qT_sb, w_sb = [], [], []
        for m in range(4):  # head pair m -> heads 2m (parts 0-63), 2m+1 (64-127)
            kt = const.tile([128, N + 1], F16, tag=f"kt{m}")
            nc.sync.dma_start(out=kt, in_=kT[2 * m : 2 * m + 2].rearrange("a b c -> (a b) c"))
            kT_sb.append(kt)
            qt = const.tile([128, NI], F16, tag=f"qt{m}")
            nc.sync.dma_start(out=qt, in_=qT[2 * m : 2 * m + 2].rearrange("a b c -> (a b) c"))
            qT_sb.append(qt)
            w = const.tile([128, F], F16, tag=f"w{m}")
            nc.sync.dma_start(out=w, in_=wT[m * 128 : (m + 1) * 128, :])
            w_sb.append(w)
        vA_sb = const.tile([128, JT * H * 65], BF16, tag="vA")
        nc.sync.dma_start(
            out=vA_sb[:, :].rearrange("p (a c) -> p a c", a=JT),
            in_=vA[0 : JT * 128, :].rearrange("(a p) c -> p a c", p=128),
        )
        vnull = const.tile([1, H * 65], BF16, tag="vnull")
        nc.sync.dma_start(out=vnull, in_=vA[N : N + 1, :])
        gam_sb = const.tile([128, F], F32, tag="gam")
        nc.sync.dma_start(out=gam_sb, in_=gam[:, :])
        id_sb = const.tile([128, 128], F16, tag="ident")
        nc.sync.dma_start(out=id_sb, in_=ident[:, :])
        m8_sb = const.tile([8, 4 * 128], F16, tag="m8")
        nc.sync.dma_start(
            out=m8_sb[:, :].rearrange("p (a c) -> p a c", a=4),
            in_=m8[:, :, :].rearrange("a p c -> p a c"),
        )
        one11 = const.tile([1, 1], F16, tag="one11")
        nc.sync.dma_start(out=one11, in_=one1[:, :])

        sums_all = smalls.tile([8, NI], F32, tag="sums_all", bufs=1)

        # PE warmup burst: keep the array busy while the first bias DMA
        # lands so HAM is at full clock when the real matmuls start.
        warm = ps_misc.tile([128, 512], F32, tag="misc", name="warm")
        for _ in range(40):
            nc.tensor.matmul(warm[:, 0:128], lhsT=id_sb, rhs=id_sb, start=True, stop=True)

        xts = {}
        for m in range(4):
            for half in range(2):
                xts[(m, half)] = xtp.tile(
                    [128, 512], F16, tag=f"xt{m}_{half}", name=f"xt{m}_{half}"
                )

        # ---- attention ---------------------------------------------------
        for m in range(4):
            for hh in range(2):
                h = 2 * m + hh
                hs = slice(hh * 64, hh * 64 + 64)
                bsb = biasp.tile([128, JT * NI], F16, tag="bias")
                nc.sync.dma_start(
                    out=bsb[:, :].rearrange("p (a f) -> p a f", a=JT),
                    in_=biasT[h, 0:N, :].rearrange("(a p) f -> p a f", p=128),
                )
                pv = [
                    ps_misc.tile([65, 512], F32, tag="misc", name=f"pv{h}_{half}")
                    for half in range(2)
                ]
                for jt in range(JT):
                    sp = ps_s.tile([128, NI], F32, tag="sp", name=f"sp{h}_{jt}")
                    for half in range(2):
                        cs = slice(half * 512, half * 512 + 512)
                        nc.tensor.matmul(
                            sp[:, cs],
                            lhsT=kT_sb[m][hs, jt * 128 : (jt + 1) * 128],
                            rhs=qT_sb[m][hs, cs],
                            start=True,
                            stop=False,
                        )
                        nc.tensor.matmul(
                            sp[:, cs],
                            lhsT=id_sb,
                            rhs=bsb[:, jt * NI + half * 512 : jt * NI + half * 512 + 512],
                            start=False,
                            stop=True,
                        )
                    pt = ptp.tile([128, NI], BF16, tag="pt")
                    nc.scalar.activation(pt, sp, ACTF.Exp)
                    for half in range(2):
                        cs = slice(half * 512, half * 512 + 512)
                        nc.tensor.matmul(
                            pv[half],
                            lhsT=vA_sb[:, (jt * H + h) * 65 : (jt * H + h + 1) * 65],
                            rhs=pt[:, cs],
                            start=(jt == 0),
                            stop=False,
                        )
                # null token (j' = 2048)
                sn = ps_s.tile([1, NI], F32, tag="sp", name=f"sn{h}")
                nb = smalls.tile([1, NI], F16, tag="nb")
                nc.sync.dma_start(out=nb, in_=biasT[h, N : N + 1, :])
                for half in range(2):
                    cs = slice(half * 512, half * 512 + 512)
                    nc.tensor.matmul(
                        sn[:, cs],
                        lhsT=kT_sb[m][hs, N : N + 1],
                        rhs=qT_sb[m][hs, cs],
                        start=True,
                        stop=False,
                    )
                    nc.tensor.matmul(
                        sn[:, cs], lhsT=one11, rhs=nb[0:1, cs], start=False, stop=True
                    )
                ptn = smalls.tile([1, NI], BF16, tag="ptn")
                nc.scalar.activation(ptn, sn, ACTF.Exp)
                for half in range(2):
                    cs = slice(half * 512, half * 512 + 512)
                    nc.tensor.matmul(
                        pv[half],
                        lhsT=vnull[0:1, h * 65 : (h + 1) * 65],
                        rhs=ptn[0:1, cs],
                        start=False,
                        stop=True,
                    )
                # stash unnormalized X^T rows; route row sums into sums_all
                for half in range(2):
                    nc.vector.tensor_copy(xts[(m, half)][hs, :], pv[half][0:64, :])
                    ssb = smalls.tile([1, 512], F32, tag="ssb")
                    nc.vector.tensor_copy(ssb, pv[half][64:65, :])
                    nc.sync.dma_start(
                        out=sums_all[h : h + 1, half * 512 : half * 512 + 512], in_=ssb
                    )
        # ---- normalize ---------------------------------------------------
        rca = smalls.tile([8, NI], F16, tag="rca", bufs=1)
        with nc.allow_low_precision(reason="1/sums broadcast is fp16 on the PE"):
            nc.vector.reciprocal(rca, sums_all)
        for m in range(4):
            for half in range(2):
                rr_ps = ps_misc.tile([128, 512], F32, tag="misc", name=f"rr{m}_{half}")
                nc.tensor.matmul(
                    rr_ps,
                    lhsT=m8_sb[:, m * 128 : (m + 1) * 128],
                    rhs=rca[0:8, half * 512 : half * 512 + 512],
                    start=True,
                    stop=True,
                )
                rr_sb = rrp.tile([128, 512], F32, tag="rr_sb")
                nc.vector.tensor_copy(rr_sb, rr_ps)
                nc.vector.tensor_mul(xts[(m, half)], xts[(m, half)], rr_sb)
        # ---- projection + CenteredLayerNorm ------------------------------
        for it in range(8):
            half, itc = it // 4, it % 4
            pp = ps_misc.tile([128, 512], F32, tag="misc", name=f"pp{it}")
            for m in range(4):
                nc.tensor.matmul(
                    pp,
                    lhsT=xts[(m, half)][:, itc * 128 : (itc + 1) * 128],
                    rhs=w_sb[m],
                    start=(m == 0),
                    stop=(m == 3),
                )
            s1 = smalls.tile([128, 1], F32, tag="s1")
            nc.vector.reduce_sum(s1, pp, axis=AX)
            mu = smalls.tile([128, 1], F32, tag="mu")
            nc.vector.tensor_scalar_mul(mu, s1, 1.0 / F)
            cen = lnp.tile([128, 512], F32, tag="cen")
            nc.vector.tensor_scalar(
                out=cen, in0=pp, scalar1=mu, scalar2=None, op0=ALU.subtract
            )
            sq = lnp.tile([128, 512], F32, tag="sq")
            var = smalls.tile([128, 1], F32, tag="var")
            nc.scalar.activation(sq, cen, ACTF.Square, accum_out=var)
            v2 = smalls.tile([128, 1], F32, tag="v2")
            nc.vector.tensor_scalar(
                out=v2, in0=var, scalar1=1.0 / F, scalar2=EPS, op0=ALU.mult, op1=ALU.add
            )
            lnv = smalls.tile([128, 1], F32, tag="lnv")
            nc.scalar.activation(lnv, v2, ACTF.Ln)
            rstd = smalls.tile([128, 1], F32, tag="rstd")
            nc.scalar.activation(rstd, lnv, ACTF.Exp, scale=-0.5)
            o1 = lnp.tile([128, 512], F32, tag="o1")
            nc.vector.tensor_scalar_mul(o1, cen, rstd)
            o2 = outpool.tile([128, 512], F32, tag="o2")
            nc.vector.tensor_mul(o2, o1, gam_sb)
            nc.sync.dma_start(out=outp[it * 128 : (it + 1) * 128, :], in_=o2)
    nc.finalize()
    return nc


def _host_prep(q, k, v, mask, bias, tokens, w_out, gamma):
    """Build the 8 per-core input maps (all plain numpy)."""
    wTc = np.ascontiguousarray(w_out.T.astype(np.float16))  # [MID, F]
    gam_rep = np.ascontiguousarray(np.broadcast_to(gamma[None, :], (128, F)))
    ident = np.eye(128, dtype=np.float16)
    m8 = np.zeros((4, 8, 128), np.float16)
    for m in range(4):
        m8[m, 2 * m, :64] = 1.0
        m8[m, 2 * m + 1, 64:] = 1.0

    in_maps = [None] * NCORES
    for b in range(B):
        ka = np.concatenate([k[b], np.tile(tokens[0], H)[None, :]], axis=0)  # [N+1, MID]
        kTb = np.ascontiguousarray(
            ka.reshape(N + 1, H, D).transpose(1, 2, 0).astype(np.float16)
        )
        va = np.concatenate([v[b], np.tile(tokens[1], H)[None, :]], axis=0)
        vAb = np.ascontiguousarray(
            np.concatenate(
                [va.reshape(N + 1, H, D), np.ones((N + 1, H, 1), np.float32)], axis=2
            )
            .reshape(N + 1, H * 65)
            .astype(ml_dtypes.bfloat16)
        )
        maskb = np.concatenate([mask[b], [True]])  # j' order: tokens..., null
        # reorder bias j: null column (orig j=0) moved to the end
        bb = np.concatenate([bias[b, :, :, 1:], bias[b, :, :, 0:1]], axis=2)
        bb = np.where(maskb[None, None, :], bb, NEG).astype(np.float16)
        bbT = bb.transpose(0, 2, 1)  # [H, N+1, N] (view)
        for half in range(2):
            c = 2 * b + half
            i0 = half * NI
            qTc = (
                q[b, i0 : i0 + NI].reshape(NI, H, D).transpose(1, 2, 0) / 8.0
            ).astype(np.float16)
            in_maps[c] = {
                "biasT": np.ascontiguousarray(bbT[:, :, i0 : i0 + NI]),
                "qT": np.ascontiguousarray(qTc),
                "kT": kTb,
                "vA": vAb,
                "wT": wTc,
                "gam": gam_rep,
                "ident": ident,
                "m8": m8,
                "one1": np.ones((1, 1), np.float16),
            }
    return in_maps


def kernel(q, k, v, mask, attention_bias, tokens, w_out, gamma):
    global LAST_RESULT
    q = np.asarray(q, np.float32)
    k = np.asarray(k, np.float32)
    v = np.asarray(v, np.float32)
    mask = np.asarray(mask, bool)
    bias = np.asarray(attention_bias, np.float32)
    tokens = np.asarray(tokens, np.float32)
    w_out = np.asarray(w_out, np.float32)
    gamma = np.asarray(gamma, np.float32)

    if "nc" not in _NC_CACHE:
        _NC_CACHE["nc"] = build_nc()
    nc = _NC_CACHE["nc"]

    in_maps = _host_prep(q, k, v, mask, bias, tokens, w_out, gamma)
    trace = os.environ.get("KERNEL_TRACE", "0") == "1"
    if trace:
        _ensure_ntff_hook()
        try:
            res = run_bass_kernel_spmd(nc, in_maps, list(range(NCORES)), trace=True)
        except Exception as e:
            print(f"trace run failed ({type(e).__name__}: {e}); retrying untraced")
            res = run_bass_kernel_spmd(nc, in_maps, list(range(NCORES)), trace=False)
    else:
        res = run_bass_kernel_spmd(nc, in_maps, list(range(NCORES)), trace=False)
    LAST_RESULT = res

    out = np.empty((B, N, F), np.float32)
    for c in range(NCORES):
        out[c // 2, (c % 2) * NI : (c % 2) * NI + NI, :] = res.results[c]["out"]
    return out


# revision 14
# speedup vs baseline: 1.9546x; 1.1438x over previous
"""Trainium2 Bass kernel for AttentionBase (b=4, n=2048, h=8, d=64, F=512).

Sharding: 8 cores; core c handles batch b = c//2, query rows
i in [(c%2)*1024, (c%2)*1024 + 1024), all 8 heads. Each core's output slice
is independent -> no collectives; host gathers by concatenation.

v2 design (per core):
  - Host pre-transposes/casts everything so the device does only dense
    1-cycle/row matmuls: biasT[h, j', i] fp16 (mask folded as -20000,
    null-token column moved to j' = 2048), qT[h, d, i] fp16 (pre-scaled by
    d^-0.5), kT[h, d, j'] fp16, vA[j', h*65] bf16 (v columns + ones column
    per head for softmax row sums), wT fp16.
  - S^T[j,i] = K @ Q^T (contraction d=64, 2 heads packed per 128-partition
    tile), bias added by accumulating I.T @ biasT into PSUM.
  - P^T = exp(S^T) on ACT straight from PSUM into bf16 (no max subtraction:
    logits are O(12); bf16 has fp32 range so exp never overflows; masked
    entries are -20000 -> exp = 0 exactly).
  - PV: lhsT = [V | ones] bf16 (M=65) so PSUM row 64 is the softmax
    denominator. Row sums of all heads are DMA-gathered into one [8,1024]
    tile for a single batched DVE reciprocal; 1/sums are broadcast across
    partitions with a small selector matmul and applied on the X^T copy.
  - Projection X @ W^T in natural [i, F] layout, then CenteredLayerNorm
    along the free dim (rstd via exp(-0.5*ln(var)) to stay in one ACT
    table set with exp).
  PSUM: 2 x [128,1024] S tiles (4 banks) + 4 shared [128,512] slots
  (PV halves / broadcast / projection) = 8 banks exactly.
"""

import os
import numpy as np
from contextlib import ExitStack

import ml_dtypes
import concourse.bass as bass
import concourse.bacc as bacc
import concourse.tile as tile
import concourse.mybir as mybir
from concourse.bass_utils import run_bass_kernel_spmd

B, N, H, D = 4, 2048, 8, 64
MID = H * D  # 512
F = 512
NCORES = 8
NI = 1024  # query rows per core
JT = 16  # full 128-row j' tiles (regular tokens); null token handled apart
EPS = 1e-5
NEG = np.float32(-20000.0)

F32 = mybir.dt.float32
F16 = mybir.dt.float16
BF16 = mybir.dt.bfloat16
AX = mybir.AxisListType.X
ALU = mybir.AluOpType
ACTF = mybir.ActivationFunctionType

LAST_RESULT = None  # BassKernelResults of the most recent run (for test.py)
_NC_CACHE = {}


def _ensure_ntff_hook():
    """Register the axon NTFF profiling hook if the image lacks antenv.axon_hooks."""
    import sys
    import types

    try:
        from antenv.axon_hooks import get_axon_ntff_profile_hook  # noqa: F401

        return
    except ImportError:
        pass
    mod = types.ModuleType("antenv.axon_hooks")
    holder = {"h": None}
    mod.set_axon_ntff_profile_hook = lambda h: holder.__setitem__("h", h)
    mod.get_axon_ntff_profile_hook = lambda: holder["h"]
    import antenv

    sys.modules["antenv.axon_hooks"] = mod
    antenv.axon_hooks = mod
    try:
        from trn_agent_boot.trn_boot import _ntff_profile_via_ctypes

        h = _ntff_profile_via_ctypes("/opt/axon/libaxon_pjrt.so")
        if h is not None:
            mod.set_axon_ntff_profile_hook(h)
    except Exception:
        pass


def build_nc():
    nc = bacc.Bacc()
    biasT = nc.declare_dram_parameter("biasT", [H, N + 1, NI], F16, isOutput=False)
    qT = nc.declare_dram_parameter("qT", [H, D, NI], F16, isOutput=False)
    kT = nc.declare_dram_parameter("kT", [H, D, N + 1], F16, isOutput=False)
    vA = nc.declare_dram_parameter("vA", [N + 1, H * 65], BF16, isOutput=False)
    wT = nc.declare_dram_parameter("wT", [MID, F], F16, isOutput=False)
    gam = nc.declare_dram_parameter("gam", [128, F], F32, isOutput=False)
    ident = nc.declare_dram_parameter("ident", [128, 128], F16, isOutput=False)
    m8 = nc.declare_dram_parameter("m8", [4, 8, 128], F16, isOutput=False)
    one1 = nc.declare_dram_parameter("one1", [1, 1], F16, isOutput=False)
    outp = nc.declare_dram_parameter("out", [NI, F], F32, isOutput=True)

    with ExitStack() as ctx:
        tc = ctx.enter_context(tile.TileContext(nc))
        const = ctx.enter_context(tc.tile_pool(name="const", bufs=1))
        biasp = ctx.enter_context(tc.tile_pool(name="biasp", bufs=4))
        ptp = ctx.enter_context(tc.tile_pool(name="ptp", bufs=4))
        smalls = ctx.enter_context(tc.tile_pool(name="smalls", bufs=2))
        xtp = ctx.enter_context(tc.tile_pool(name="xtp", bufs=1))
        rrp = ctx.enter_context(tc.tile_pool(name="rrp", bufs=2))
        lnp = ctx.enter_context(tc.tile_pool(name="lnp", bufs=2))
        outpool = ctx.enter_context(tc.tile_pool(name="outpool", bufs=3))
        ps_s = ctx.enter_context(tc.tile_pool(name="ps_s", bufs=2, space="PSUM"))
        ps_misc = ctx.enter_context(tc.tile_pool(name="ps_misc", bufs=4, space="PSUM"))

        # ---- persistent tiles (DMAs emitted just-in-time below) ----------
        kT_sb = [const.tile([128, N + 1], F16, tag=f"kt{m}", name=f"kt{m}") for m in range(4)]
        qT_sb = [const.tile([128, NI], F16, tag=f"qt{m}", name=f"qt{m}") for m in range(4)]
        w_sb = [const.tile([128, F], F16, tag=f"w{m}", name=f"w{m}") for m in range(4)]
        vA_sb = const.tile([128, JT * H * 65], BF16, tag="vA")
        vnull = const.tile([1, H * 65], BF16, tag="vnull")
        gam_sb = const.tile([128, F], F32, tag="gam")
        id_sb = const.tile([128, 128], F16, tag="ident")
        m8_sb = const.tile([8, 4 * 128], F16, tag="m8")
        one11 = const.tile([1, 1], F16, tag="one11")
        sums_all = smalls.tile([8, NI], F32, tag="sums_all", bufs=1)

        def load_pair(m):
            nc.sync.dma_start(
                out=kT_sb[m], in_=kT[2 * m : 2 * m + 2].rearrange("a b c -> (a b) c")
            )
            nc.sync.dma_start(
                out=qT_sb[m], in_=qT[2 * m : 2 * m + 2].rearrange("a b c -> (a b) c")
            )

        bias_tiles = {}

        def load_bias(h):
            # two 2 MB chunks: jt 0-7 and jt 8-15
            ts = []
            for c in range(2):
                t = biasp.tile([128, 8 * NI], F16, tag="bias", name=f"bias{h}_{c}")
                nc.sync.dma_start(
                    out=t[:, :].rearrange("p (a f) -> p a f", a=8),
                    in_=biasT[h, c * 1024 : (c + 1) * 1024, :].rearrange(
                        "(a p) f -> p a f", p=128
                    ),
                )
                ts.append(t)
            bias_tiles[h] = ts

        # DMA order: identity (warmup dep) -> pair0 K/Q -> head0 bias A ->
        # vA -> head0 bias B -> remaining consts. Sync FIFO executes in
        # program order, so the first QK can start ~13us in.
        nc.sync.dma_start(out=id_sb, in_=ident[:, :])
        # PE warmup burst: keep the array busy until real matmuls start.
        warm = ps_misc.tile([128, 512], F32, tag="misc", name="warm")
        for _ in range(120):
            nc.tensor.matmul(warm[:, 0:128], lhsT=id_sb, rhs=id_sb, start=True, stop=True)
        load_pair(0)
        load_bias(0)
        nc.sync.dma_start(
            out=vA_sb[:, :].rearrange("p (a c) -> p a c", a=JT),
            in_=vA[0 : JT * 128, :].rearrange("(a p) c -> p a c", p=128),
        )
        nc.sync.dma_start(out=vnull, in_=vA[N : N + 1, :])
        for m in range(4):
            nc.sync.dma_start(out=w_sb[m], in_=wT[m * 128 : (m + 1) * 128, :])
        nc.sync.dma_start(out=gam_sb, in_=gam[:, :])
        nc.sync.dma_start(
            out=m8_sb[:, :].rearrange("p (a c) -> p a c", a=4),
            in_=m8[:, :, :].rearrange("a p c -> p a c"),
        )
        nc.sync.dma_start(out=one11, in_=one1[:, :])

        xts = {}
        for m in range(4):
            for half in range(2):
                xts[(m, half)] = xtp.tile(
                    [128, 512], F16, tag=f"xt{m}_{half}", name=f"xt{m}_{half}"
                )

        # ---- attention ---------------------------------------------------
        for m in range(4):
            for hh in range(2):
                h = 2 * m + hh
                hs = slice(hh * 64, hh * 64 + 64)
                # prefetch next head's inputs (Sync FIFO order = issue order)
                if h + 1 < H:
                    if (h + 1) % 2 == 0:
                        load_pair((h + 1) // 2)
                    load_bias(h + 1)
                bsA, bsB = bias_tiles.pop(h)
                pv = [
                    ps_misc.tile([65, 512], F32, tag="misc", name=f"pv{h}_{half}")
                    for half in range(2)
                ]
                for jt in range(JT):
                    bsb = bsA if jt < 8 else bsB
                    jo = (jt % 8) * NI
                    sp = ps_s.tile([128, NI], F32, tag="sp", name=f"sp{h}_{jt}")
                    for half in range(2):
                        cs = slice(half * 512, half * 512 + 512)
                        nc.tensor.matmul(
                            sp[:, cs],
                            lhsT=kT_sb[m][hs, jt * 128 : (jt + 1) * 128],
                            rhs=qT_sb[m][hs, cs],
                            start=True,
                            stop=False,
                        )
                        nc.tensor.matmul(
                            sp[:, cs],
                            lhsT=id_sb,
                            rhs=bsb[:, jo + half * 512 : jo + half * 512 + 512],
                            start=False,
                            stop=True,
                        )
                    pt = ptp.tile([128, NI], BF16, tag="pt")
                    nc.scalar.activation(pt, sp, ACTF.Exp)
                    for half in range(2):
                        cs = slice(half * 512, half * 512 + 512)
                        nc.tensor.matmul(
                            pv[half],
                            lhsT=vA_sb[:, (jt * H + h) * 65 : (jt * H + h + 1) * 65],
                            rhs=pt[:, cs],
                            start=(jt == 0),
                            stop=False,
                        )
                # null token (j' = 2048)
                sn = ps_s.tile([1, NI], F32, tag="sp", name=f"sn{h}")
                nb = smalls.tile([1, NI], F16, tag="nb")
                nc.sync.dma_start(out=nb, in_=biasT[h, N : N + 1, :])
                for half in range(2):
                    cs = slice(half * 512, half * 512 + 512)
                    nc.tensor.matmul(
                        sn[:, cs],
                        lhsT=kT_sb[m][hs, N : N + 1],
                        rhs=qT_sb[m][hs, cs],
                        start=True,
                        stop=False,
                    )
                    nc.tensor.matmul(
                        sn[:, cs], lhsT=one11, rhs=nb[0:1, cs], start=False, stop=True
                    )
                ptn = smalls.tile([1, NI], BF16, tag="ptn")
                nc.scalar.activation(ptn, sn, ACTF.Exp)
                for half in range(2):
                    cs = slice(half * 512, half * 512 + 512)
                    nc.tensor.matmul(
                        pv[half],
                        lhsT=vnull[0:1, h * 65 : (h + 1) * 65],
                        rhs=ptn[0:1, cs],
                        start=False,
                        stop=True,
                    )
                # stash unnormalized X^T rows; route row sums into sums_all
                for half in range(2):
                    nc.vector.tensor_copy(xts[(m, half)][hs, :], pv[half][0:64, :])
                    ssb = smalls.tile([1, 512], F32, tag="ssb")
                    nc.vector.tensor_copy(ssb, pv[half][64:65, :])
                    nc.sync.dma_start(
                        out=sums_all[h : h + 1, half * 512 : half * 512 + 512], in_=ssb
                    )
        # ---- normalize ---------------------------------------------------
        rca = smalls.tile([8, NI], F16, tag="rca", bufs=1)
        with nc.allow_low_precision(reason="1/sums broadcast is fp16 on the PE"):
            nc.vector.reciprocal(rca, sums_all)
        for m in range(4):
            for half in range(2):
                rr_ps = ps_misc.tile([128, 512], F32, tag="misc", name=f"rr{m}_{half}")
                nc.tensor.matmul(
                    rr_ps,
                    lhsT=m8_sb[:, m * 128 : (m + 1) * 128],
                    rhs=rca[0:8, half * 512 : half * 512 + 512],
                    start=True,
                    stop=True,
                )
                rr_sb = rrp.tile([128, 512], F32, tag="rr_sb")
                nc.vector.tensor_copy(rr_sb, rr_ps)
                nc.vector.tensor_mul(xts[(m, half)], xts[(m, half)], rr_sb)
        # ---- projection + CenteredLayerNorm ------------------------------
        for it in range(8):
            half, itc = it // 4, it % 4
            pp = ps_misc.tile([128, 512], F32, tag="misc", name=f"pp{it}")
            for m in range(4):
                nc.tensor.matmul(
                    pp,
                    lhsT=xts[(m, half)][:, itc * 128 : (itc + 1) * 128],
                    rhs=w_sb[m],
                    start=(m == 0),
                    stop=(m == 3),
                )
            s1 = smalls.tile([128, 1], F32, tag="s1")
            nc.vector.reduce_sum(s1, pp, axis=AX)
            mu = smalls.tile([128, 1], F32, tag="mu")
            nc.vector.tensor_scalar_mul(mu, s1, 1.0 / F)
            cen = lnp.tile([128, 512], F32, tag="cen")
            nc.vector.tensor_scalar(
                out=cen, in0=pp, scalar1=mu, scalar2=None, op0=ALU.subtract
            )
            sq = lnp.tile([128, 512], F32, tag="sq")
            var = smalls.tile([128, 1], F32, tag="var")
            nc.scalar.activation(sq, cen, ACTF.Square, accum_out=var)
            v2 = smalls.tile([128, 1], F32, tag="v2")
            nc.vector.tensor_scalar(
                out=v2, in0=var, scalar1=1.0 / F, scalar2=EPS, op0=ALU.mult, op1=ALU.add
            )
            lnv = smalls.tile([128, 1], F32, tag="lnv")
            nc.scalar.activation(lnv, v2, ACTF.Ln)
            rstd = smalls.tile([128, 1], F32, tag="rstd")
            nc.scalar.activation(rstd, lnv, ACTF.Exp, scale=-0.5)
            o1 = lnp.tile([128, 512], F32, tag="o1")
            nc.vector.tensor_scalar_mul(o1, cen, rstd)
            o2 = outpool.tile([128, 512], F32, tag="o2")
            nc.vector.tensor_mul(o2, o1, gam_sb)
            nc.sync.dma_start(out=outp[it * 128 : (it + 1) * 128, :], in_=o2)
    nc.finalize()
    return nc


def _host_prep(q, k, v, mask, bias, tokens, w_out, gamma):
    """Build the 8 per-core input maps (all plain numpy)."""
    wTc = np.ascontiguousarray(w_out.T.astype(np.float16))  # [MID, F]
    gam_rep = np.ascontiguousarray(np.broadcast_to(gamma[None, :], (128, F)))
    ident = np.eye(128, dtype=np.float16)
    m8 = np.zeros((4, 8, 128), np.float16)
    for m in range(4):
        m8[m, 2 * m, :64] = 1.0
        m8[m, 2 * m + 1, 64:] = 1.0

    in_maps = [None] * NCORES
    for b in range(B):
        ka = np.concatenate([k[b], np.tile(tokens[0], H)[None, :]], axis=0)  # [N+1, MID]
        kTb = np.ascontiguousarray(
            ka.reshape(N + 1, H, D).transpose(1, 2, 0).astype(np.float16)
        )
        va = np.concatenate([v[b], np.tile(tokens[1], H)[None, :]], axis=0)
        vAb = np.ascontiguousarray(
            np.concatenate(
                [va.reshape(N + 1, H, D), np.ones((N + 1, H, 1), np.float32)], axis=2
            )
            .reshape(N + 1, H * 65)
            .astype(ml_dtypes.bfloat16)
        )
        maskb = np.concatenate([mask[b], [True]])  # j' order: tokens..., null
        # reorder bias j: null column (orig j=0) moved to the end
        bb = np.concatenate([bias[b, :, :, 1:], bias[b, :, :, 0:1]], axis=2)
        bb = np.where(maskb[None, None, :], bb, NEG).astype(np.float16)
        bbT = bb.transpose(0, 2, 1)  # [H, N+1, N] (view)
        for half in range(2):
            c = 2 * b + half
            i0 = half * NI
            qTc = (
                q[b, i0 : i0 + NI].reshape(NI, H, D).transpose(1, 2, 0) / 8.0
            ).astype(np.float16)
            in_maps[c] = {
                "biasT": np.ascontiguousarray(bbT[:, :, i0 : i0 + NI]),
                "qT": np.ascontiguousarray(qTc),
                "kT": kTb,
                "vA": vAb,
                "wT": wTc,
                "gam": gam_rep,
                "ident": ident,
                "m8": m8,
                "one1": np.ones((1, 1), np.float16),
            }
    return in_maps


def kernel(q, k, v, mask, attention_bias, tokens, w_out, gamma):
    global LAST_RESULT
    q = np.asarray(q, np.float32)
    k = np.asarray(k, np.float32)
    v = np.asarray(v, np.float32)
    mask = np.asarray(mask, bool)
    bias = np.asarray(attention_bias, np.float32)
    tokens = np.asarray(tokens, np.float32)
    w_out = np.asarray(w_out, np.float32)
    gamma = np.asarray(gamma, np.float32)

    if "nc" not in _NC_CACHE:
        _NC_CACHE["nc"] = build_nc()
    nc = _NC_CACHE["nc"]

    in_maps = _host_prep(q, k, v, mask, bias, tokens, w_out, gamma)
    trace = os.environ.get("KERNEL_TRACE", "0") == "1"
    if trace:
        _ensure_ntff_hook()
        try:
            res = run_bass_kernel_spmd(nc, in_maps, list(range(NCORES)), trace=True)
        except Exception as e:
            print(f"trace run failed ({type(e).__name__}: {e}); retrying untraced")
            res = run_bass_kernel_spmd(nc, in_maps, list(range(NCORES)), trace=False)
    else:
        res = run_bass_kernel_spmd(nc, in_maps, list(range(NCORES)), trace=False)
    LAST_RESULT = res

    out = np.empty((B, N, F), np.float32)
    for c in range(NCORES):
        out[c // 2, (c % 2) * NI : (c % 2) * NI + NI, :] = res.results[c]["out"]
    return out
